# revision 6
# baseline (speedup 1.0000x reference)
"""BoundaryMaxPooling Trainium2 kernel, v2.

Sharding: channel-parallel. Core k owns channels [128k, 128k+128) for ALL
8 batches (all batches share batch-0 segment windows, so each core's 128
channels see ONE window set: cores 0-3 the 'start' half, 4-7 the 'end').

Algorithm: sparse-table RMQ in bf16 with the 8 batches interleaved as
lanes ([c, t, 8lane] per partition). Levels are built by doubling with
FLAT contiguous tensor_tensor max (shift by s positions == shift by 8s
elements). Only 4 level rows are live at a time (row r holds level k with
k%4==r); level k+4 overwrites row r after level-k gathers finish (the
tile framework's WAR tracking enforces this).

Queries are sorted by level k=floor(log2(len)) on the host; each level's
(idx1, idx2) pairs are gathered from that level's row with ONE ap_gather
(d=8 lanes, interleaved pairs), folded pairwise with a strided DVE max,
and DMAed out. k=8 queries (len>=256) expand to two level-7 slots merged
on the host. The host applies the inverse permutation + lane de-interleave.
"""

import numpy as np
import ml_dtypes

B, C2, T = 8, 1024, 2048
P = 128
N_CORES = 8
LANES = 8
CHUNK = 256           # max slots per gather chunk
# big levels are split into two position ranges, each built into its OWN
# row tile (so gathers on one range never WAR-block the other's build).
# level -> list of (i1_range_end, build_lo, build_hi, dst_row)
# row cycle: A: x,l4,l7lo  B: l1,l5  C: l2,l6lo  D: l3,l6hi  E(4): l7hi
PS_RANGES = {6: [(1024, 0, 1216, 2), (2048, 1024, 1985, 3)],
             7: [(1024, 0, 1152, 0), (2048, 1024, 1921, 4)]}
L7HI_W = 1921 - 1024   # fifth tile width (cols)
BF16 = ml_dtypes.bfloat16

_CACHE = {}


# ---------------------------------------------------------------- host math
def _windows(seg0, max_len):
    """lo, hi per query for both halves; exact reference arithmetic."""
    seg0 = np.clip(seg0.astype(np.float64), 0.0, float(max_len - 1))
    out = []
    for c0, c1 in ((0, 1), (2, 3)):
        lo = np.floor(seg0[:, c0]).astype(np.int64)
        hi = np.ceil(seg0[:, c1]).astype(np.int64)
        hi = np.maximum(hi, lo + 1)
        out.append((lo, hi))
    return out


def _plan_half(lo, hi):
    """Build per-level slot lists for one half.

    Returns:
      levels: list over k=0..7 of (i1 array, i2 array, q array)
              (k=8 queries contribute two level-7 slots with same q)
    """
    L = hi - lo
    k = np.array([int(x).bit_length() - 1 for x in L], dtype=np.int64)
    levels = []
    for kk in range(8):
        sel = np.nonzero(k == kk)[0]
        i1 = lo[sel]
        i2 = hi[sel] - (1 << kk)
        levels.append([i1, i2, sel])
    # k == 8 -> two level-7 slots each
    sel8 = np.nonzero(k >= 8)[0]
    if len(sel8):
        i1a, i2a = lo[sel8], lo[sel8] + 128
        i1b, i2b = hi[sel8] - 256, hi[sel8] - 128
        l7 = levels[7]
        levels[7] = [
            np.concatenate([l7[0], i1a, i1b]),
            np.concatenate([l7[1], i2a, i2b]),
            np.concatenate([l7[2], sel8, sel8]),
        ]
    # split into groups: one per level for k<=5; position-ranges (by i1,
    # sorted) for the big levels so gathers can chase the col-chunked build.
    # Group order == device emission order (level-5 gathers run LAST: its
    # source row is never recycled, so the big levels get GPSIMD priority).
    gmap = {}
    for kk in range(8):
        i1, i2, q = levels[kk]
        if kk not in PS_RANGES:
            gmap[(kk, 0)] = [(kk, 0), i1, i2, q]
            continue
        o = np.argsort(i1, kind="stable")
        i1, i2, q = i1[o], i2[o], q[o]
        lo_r = 0
        for ri, (rend, blo, bhi, _row) in enumerate(PS_RANGES[kk]):
            hi_r = np.searchsorted(i1, rend, side="left")
            # indices are rebased to the sub-tile's local origin (blo)
            gmap[(kk, ri)] = [(kk, ri), i1[lo_r:hi_r] - blo,
                              i2[lo_r:hi_r] - blo, q[lo_r:hi_r]]
            lo_r = hi_r
    order = [(0, 0), (1, 0), (2, 0), (3, 0), (4, 0),
             (6, 0), (6, 1), (7, 0), (7, 1), (5, 0)]
    return [gmap[k] for k in order]


def _uniform_plan(groups_s, groups_e):
    """Pad both halves' groups to common slot counts (mult of 16), split
    into chunks of <= CHUNK slots. Returns list of (group_key, n_slots)."""
    plan = []
    for gs, ge in zip(groups_s, groups_e):
        assert gs[0] == ge[0]
        n = max(len(gs[1]), len(ge[1]))
        n = max(((n + 15) // 16) * 16, 16)
        while n > 0:
            c = min(n, CHUNK)
            plan.append((gs[0], c))
            n -= c
    return plan


def _wrap(idx):
    blk = np.asarray(idx, dtype=np.int16).reshape(-1, 16).T  # [16, n/16]
    return np.tile(blk, (8, 1))                              # [128, n/16]


def _half_streams(groups, plan):
    """Per-half: build wrapped idx tensor + slot bookkeeping."""
    gmap = {g[0]: g for g in groups}
    nslot_total = sum(c for _, c in plan)
    idx_cols = []
    qmap = np.full(nslot_total, -1, dtype=np.int64)
    off = 0
    consumed = {g[0]: 0 for g in groups}
    for key, c in plan:
        _, i1, i2, q = gmap[key]
        s = consumed[key]
        take = min(max(len(i1) - s, 0), c)
        ii1 = np.zeros(c, dtype=np.int64)
        ii2 = np.zeros(c, dtype=np.int64)
        ii1[:take] = i1[s:s + take]
        ii2[:take] = i2[s:s + take]
        qmap[off:off + take] = q[s:s + take]
        consumed[key] = s + take
        inter = np.empty(2 * c, dtype=np.int64)
        inter[0::2] = ii1
        inter[1::2] = ii2
        idx_cols.append(_wrap(inter))
        off += c
    idxw = np.concatenate(idx_cols, axis=1)  # [128, 2*nslot/16]
    return idxw.astype(np.int16), qmap


# ---------------------------------------------------------------- program
def _build_program(plan):
    import concourse.bacc as bacc
    import concourse.mybir as mybir
    import concourse.tile as tile
    from concourse.ap import AP

    bf16 = mybir.dt.bfloat16
    i16 = mybir.dt.int16
    MAX = mybir.AluOpType.max

    nslot = sum(c for _, c in plan)
    nidxcol = 2 * nslot // 16

    nc = bacc.Bacc("TRN2", target_bir_lowering=False, debug=False,
                   num_devices=N_CORES)
    xb_d = nc.dram_tensor("xb", [P, T * LANES], bf16, kind="ExternalInput")
    idx_d = nc.dram_tensor("idxw", [P, nidxcol], i16, kind="ExternalInput")
    out_d = nc.dram_tensor("out", [P, nslot * LANES], bf16,
                           kind="ExternalOutput")

    with tile.TileContext(nc) as tc:
        with tc.tile_pool(name="rows", bufs=1) as rp, \
             tc.tile_pool(name="r12", bufs=6) as gp, \
             tc.tile_pool(name="fo", bufs=3) as fp, \
             tc.tile_pool(name="ix", bufs=1) as ip:

            rows = [rp.tile([P, T * LANES], bf16, tag=f"row{r}",
                            name=f"row{r}") for r in range(4)]
            rows.append(rp.tile([P, L7HI_W * LANES], bf16, tag="row4",
                                name="row4"))
            idxt = ip.tile([P, nidxcol], i16, tag="idx", name="idxt")
            nc.sync.dma_start(idxt[:], idx_d[:])
            # init tails the level builds leave unwritten (levels read/write
            # only positions <= T - 2^k; sim-visible gather views span T)
            for r in (1, 2, 3):
                nc.vector.memset(rows[r][:, (T - 128) * LANES:], 0)

            def pdim(ap):
                return [list(p) for p in ap.ap][0]

            # input arrives pre-interleaved [c, t, lane] from the host;
            # chunked DMA so the l1 build can start early
            r0 = rows[0][:]
            ndma = 4
            dstep = T * LANES // ndma
            for j in range(ndma):
                nc.sync.dma_start(r0[:, j * dstep:(j + 1) * dstep],
                                  xb_d[:, j * dstep:(j + 1) * dstep])

            # emission helpers -------------------------------------------
            chunks_by_group = {}
            for key, c in plan:
                chunks_by_group.setdefault(key, []).append(c)

            state = {"slot": 0, "col": 0}
            pending_folds = []

            def emit_fold(item):
                r12, c, off = item
                fo = fp.tile([P, CHUNK * LANES], bf16, tag="fo", name="fo")
                v = r12[:]
                in0 = AP(v.tensor, v.offset,
                         [pdim(v), [2 * LANES, c], [1, LANES]])
                in1 = AP(v.tensor, v.offset + LANES,
                         [pdim(v), [2 * LANES, c], [1, LANES]])
                nc.vector.tensor_tensor(fo[:, :c * LANES], in0, in1, MAX)
                nc.sync.dma_start(
                    out_d[:, off * LANES:(off + c) * LANES],
                    fo[:, :c * LANES])

            def emit_group(key, eager_fold, row=None, width=T):
                if row is None:
                    row = key[0] % 4
                for c in chunks_by_group.get(key, []):
                    r12 = gp.tile([P, 2 * CHUNK * LANES], bf16, tag="r12",
                                  name="r12")
                    n_idx = 2 * c
                    src = rows[row][:]
                    src3 = AP(src.tensor, src.offset,
                              [pdim(src), [LANES, width], [1, LANES]])
                    dst3 = AP(r12[:].tensor, r12[:].offset,
                              [pdim(r12[:]), [LANES, n_idx], [1, LANES]])
                    co = state["col"]
                    nc.gpsimd.ap_gather(
                        dst3, src3, idxt[:, co:co + n_idx // 16],
                        channels=P, num_elems=width, d=LANES, num_idxs=n_idx)
                    item = (r12, c, state["slot"])
                    if eager_fold:
                        emit_fold(item)
                    else:
                        pending_folds.append(item)
                    state["slot"] += c
                    state["col"] += n_idx // 16

            emit_group((0, 0), eager_fold=False)

            for kk in range(1, 6):
                s = 1 << (kk - 1)
                npos = T - 2 * s + 1
                src = rows[(kk - 1) % 4][:]
                dst = rows[kk % 4][:]
                if kk == 1:
                    # col-chunked so each piece starts as its DMA lands
                    step = T // ndma
                    for j in range(ndma):
                        a = j * step
                        b = min((j + 1) * step, npos)
                        nc.vector.tensor_tensor(
                            dst[:, a * LANES:b * LANES],
                            src[:, a * LANES:b * LANES],
                            src[:, (a + s) * LANES:(b + s) * LANES],
                            MAX)
                else:
                    w = npos * LANES
                    nc.vector.tensor_tensor(
                        dst[:, :w], src[:, :w],
                        src[:, s * LANES:s * LANES + w], MAX)
                if kk <= 4:
                    emit_group((kk, 0), eager_fold=False)

            # big levels: each position-range built into its own row tile;
            # gathers on one range never WAR-block later builds. Level 6
            # reads l5 (row1); level 7 reads the l6 sub-tiles locally.
            big = []
            for ri, (_, blo, bhi, drow) in enumerate(PS_RANGES[6]):
                s = 32
                w = bhi - blo
                src = rows[1][:]          # l5
                dst = rows[drow][:]
                nc.vector.tensor_tensor(
                    dst[:, :w * LANES],
                    src[:, blo * LANES:(blo + w) * LANES],
                    src[:, (blo + s) * LANES:(blo + s + w) * LANES], MAX)
                emit_group((6, ri), eager_fold=False, row=drow, width=w)
            for ri, (_, blo, bhi, drow) in enumerate(PS_RANGES[7]):
                s = 64
                w = bhi - blo
                srow, sblo, sw = (2, 0, 1216) if ri == 0 else (3, 1024, 961)
                src = rows[srow][:]       # l6 sub-tile, local origin sblo
                dst = rows[drow][:]
                loc = blo - sblo
                assert loc >= 0 and loc + s + w <= sw
                nc.vector.tensor_tensor(
                    dst[:, :w * LANES],
                    src[:, loc * LANES:(loc + w) * LANES],
                    src[:, (loc + s) * LANES:(loc + s + w) * LANES], MAX)
                emit_group((7, ri), eager_fold=False, row=drow, width=w)

            # level-5 gathers last: row1 is not recycled anymore, so these
            # yield GPSIMD priority to the much larger level-6/7 groups
            emit_group((5, 0), eager_fold=False)

            for item in pending_folds:
                emit_fold(item)

    nc.compile()
    return nc


# ---------------------------------------------------------------- kernel
def _prepare(segments, max_len):
    seg0 = np.asarray(segments, dtype=np.float32)[0]
    (lo_s, hi_s), (lo_e, hi_e) = _windows(seg0, int(max_len))
    lev_s = _plan_half(lo_s, hi_s)
    lev_e = _plan_half(lo_e, hi_e)
    plan = _uniform_plan(lev_s, lev_e)
    idx_s, qmap_s = _half_streams(lev_s, plan)
    idx_e, qmap_e = _half_streams(lev_e, plan)
    return plan, (idx_s, qmap_s), (idx_e, qmap_e)


def kernel(feature, segments, max_len=T, **_unused):
    from concourse import bass_utils

    feature = np.asarray(feature)
    assert feature.shape == (B, C2, T), feature.shape
    plan, (idx_s, qmap_s), (idx_e, qmap_e) = _prepare(segments, int(max_len))

    key = ("prog", tuple(plan))
    if key not in _CACHE:
        _CACHE[key] = _build_program(plan)
    nc = _CACHE[key]

    feat_bf = feature.astype(BF16)
    in_maps = []
    for k in range(N_CORES):
        xc = feat_bf[:, k * P:(k + 1) * P, :]           # [B, 128, T]
        xb = np.ascontiguousarray(
            xc.transpose(1, 2, 0)).reshape(P, T * LANES)  # [c, t, lane]
        idxw = idx_s if k < 4 else idx_e
        in_maps.append({"xb": xb, "idxw": idxw})
    _CACHE["last_in_maps"] = in_maps

    res = bass_utils.run_bass_kernel_spmd(
        nc, in_maps, core_ids=list(range(N_CORES)))

    nslot = sum(c for _, c in plan)
    out = np.empty((B, C2, T), dtype=np.float32)
    for k in range(N_CORES):
        v = np.asarray(res.results[k]["out"]).reshape(P, nslot, LANES)
        qmap = qmap_s if k < 4 else qmap_e
        oq = _unpermute(v, qmap)          # [P, T, LANES] float32
        out[:, k * P:(k + 1) * P, :] = oq.transpose(2, 0, 1)
    return out


def _unpermute(v, qmap):
    """v: [P, nslot, LANES] bf16; qmap: slot -> q (-1 pads, dup q twice)."""
    vf = v.astype(np.float32)
    out = np.empty((P, T, LANES), dtype=np.float32)
    sl = np.nonzero(qmap >= 0)[0]
    qs = qmap[sl]
    out[:, qs, :] = vf[:, sl, :]
    # k=8 queries occupy two slots with the same q: max-combine those few
    uq, first, cnt = np.unique(qs, return_index=True, return_counts=True)
    for q in uq[cnt > 1]:
        ss = sl[qs == q]
        out[:, q, :] = vf[:, ss, :].max(axis=1)
    return out


# revision 7
# speedup vs baseline: 1.0313x; 1.0313x over previous
"""BoundaryMaxPooling Trainium2 kernel, v2.

Sharding: channel-parallel. Core k owns channels [128k, 128k+128) for ALL
8 batches (all batches share batch-0 segment windows, so each core's 128
channels see ONE window set: cores 0-3 the 'start' half, 4-7 the 'end').

Algorithm: sparse-table RMQ in bf16 with the 8 batches interleaved as
lanes ([c, t, 8lane] per partition). Levels are built by doubling with
FLAT contiguous tensor_tensor max (shift by s positions == shift by 8s
elements). Only 4 level rows are live at a time (row r holds level k with
k%4==r); level k+4 overwrites row r after level-k gathers finish (the
tile framework's WAR tracking enforces this).

Queries are sorted by level k=floor(log2(len)) on the host; each level's
(idx1, idx2) pairs are gathered from that level's row with ONE ap_gather
(d=8 lanes, interleaved pairs), folded pairwise with a strided DVE max,
and DMAed out. k=8 queries (len>=256) expand to two level-7 slots merged
on the host. The host applies the inverse permutation + lane de-interleave.
"""

import numpy as np
import ml_dtypes

B, C2, T = 8, 1024, 2048
P = 128
N_CORES = 8
LANES = 8
CHUNK = 2048          # max slots per gather chunk (1 chunk per group)
# big levels are split into two position ranges, each built into its OWN
# row tile (so gathers on one range never WAR-block the other's build).
# level -> list of (i1_range_end, build_lo, build_hi, dst_row)
# row cycle: A: x,l4,l7lo  B: l1,l5  C: l2,l6lo  D: l3,l6hi  E(4): l7hi
PS_RANGES = {6: [(1024, 0, 1216, 2), (2048, 1024, 1985, 3)],
             7: [(1024, 0, 1152, 0), (2048, 1024, 1921, 4)]}
L7HI_W = 1921 - 1024   # fifth tile width (cols)
BF16 = ml_dtypes.bfloat16

_CACHE = {}


# ---------------------------------------------------------------- host math
def _windows(seg0, max_len):
    """lo, hi per query for both halves; exact reference arithmetic."""
    seg0 = np.clip(seg0.astype(np.float64), 0.0, float(max_len - 1))
    out = []
    for c0, c1 in ((0, 1), (2, 3)):
        lo = np.floor(seg0[:, c0]).astype(np.int64)
        hi = np.ceil(seg0[:, c1]).astype(np.int64)
        hi = np.maximum(hi, lo + 1)
        out.append((lo, hi))
    return out


def _plan_half(lo, hi):
    """Build per-level slot lists for one half.

    Returns:
      levels: list over k=0..7 of (i1 array, i2 array, q array)
              (k=8 queries contribute two level-7 slots with same q)
    """
    L = hi - lo
    k = np.array([int(x).bit_length() - 1 for x in L], dtype=np.int64)
    levels = []
    for kk in range(8):
        sel = np.nonzero(k == kk)[0]
        i1 = lo[sel]
        i2 = hi[sel] - (1 << kk)
        levels.append([i1, i2, sel])
    # k == 8 -> two level-7 slots each
    sel8 = np.nonzero(k >= 8)[0]
    if len(sel8):
        i1a, i2a = lo[sel8], lo[sel8] + 128
        i1b, i2b = hi[sel8] - 256, hi[sel8] - 128
        l7 = levels[7]
        levels[7] = [
            np.concatenate([l7[0], i1a, i1b]),
            np.concatenate([l7[1], i2a, i2b]),
            np.concatenate([l7[2], sel8, sel8]),
        ]
    # split into groups: one per level for k<=5; position-ranges (by i1,
    # sorted) for the big levels so gathers can chase the col-chunked build.
    # Group order == device emission order (level-5 gathers run LAST: its
    # source row is never recycled, so the big levels get GPSIMD priority).
    gmap = {}
    for kk in range(8):
        i1, i2, q = levels[kk]
        if kk not in PS_RANGES:
            gmap[(kk, 0)] = [(kk, 0), i1, i2, q]
            continue
        o = np.argsort(i1, kind="stable")
        i1, i2, q = i1[o], i2[o], q[o]
        lo_r = 0
        for ri, (rend, blo, bhi, _row) in enumerate(PS_RANGES[kk]):
            hi_r = np.searchsorted(i1, rend, side="left")
            # indices are rebased to the sub-tile's local origin (blo)
            gmap[(kk, ri)] = [(kk, ri), i1[lo_r:hi_r] - blo,
                              i2[lo_r:hi_r] - blo, q[lo_r:hi_r]]
            lo_r = hi_r
    # groups are gathered in MERGED pairs (one ap_gather per pair) from a
    # 2-region window of the big table tile; the second group of each pair
    # gets a +T index offset to select the window's second region
    order = [(0, 0), (1, 0), (2, 0), (3, 0), (4, 0), (5, 0),
             (6, 0), (6, 1), (7, 0), (7, 1)]
    for k in ((1, 0), (3, 0), (5, 0), (6, 1), (7, 1)):
        gmap[k][1] = gmap[k][1] + T
        gmap[k][2] = gmap[k][2] + T
    return [gmap[k] for k in order]


def _uniform_plan(groups_s, groups_e):
    """Pad both halves' groups to common slot counts (mult of 16), split
    into chunks of <= CHUNK slots. Returns list of (group_key, n_slots)."""
    plan = []
    for gs, ge in zip(groups_s, groups_e):
        assert gs[0] == ge[0]
        n = max(len(gs[1]), len(ge[1]))
        n = max(((n + 15) // 16) * 16, 16)
        while n > 0:
            c = min(n, CHUNK)
            plan.append((gs[0], c))
            n -= c
    return plan


def _wrap(idx):
    blk = np.asarray(idx, dtype=np.int16).reshape(-1, 16).T  # [16, n/16]
    return np.tile(blk, (8, 1))                              # [128, n/16]


def _half_streams(groups, plan):
    """Per-half: build wrapped idx tensor + slot bookkeeping."""
    gmap = {g[0]: g for g in groups}
    nslot_total = sum(c for _, c in plan)
    idx_cols = []
    qmap = np.full(nslot_total, -1, dtype=np.int64)
    off = 0
    consumed = {g[0]: 0 for g in groups}
    for key, c in plan:
        _, i1, i2, q = gmap[key]
        s = consumed[key]
        take = min(max(len(i1) - s, 0), c)
        ii1 = np.zeros(c, dtype=np.int64)
        ii2 = np.zeros(c, dtype=np.int64)
        ii1[:take] = i1[s:s + take]
        ii2[:take] = i2[s:s + take]
        qmap[off:off + take] = q[s:s + take]
        consumed[key] = s + take
        inter = np.empty(2 * c, dtype=np.int64)
        inter[0::2] = ii1
        inter[1::2] = ii2
        idx_cols.append(_wrap(inter))
        off += c
    idxw = np.concatenate(idx_cols, axis=1)  # [128, 2*nslot/16]
    return idxw.astype(np.int16), qmap


# ---------------------------------------------------------------- program
def _build_program(plan):
    import concourse.bacc as bacc
    import concourse.mybir as mybir
    import concourse.tile as tile
    from concourse.ap import AP

    bf16 = mybir.dt.bfloat16
    i16 = mybir.dt.int16
    MAX = mybir.AluOpType.max

    nslot = sum(c for _, c in plan)
    nidxcol = 2 * nslot // 16

    nc = bacc.Bacc("TRN2", target_bir_lowering=False, debug=False,
                   num_devices=N_CORES)
    xb_d = nc.dram_tensor("xb", [P, T * LANES], bf16, kind="ExternalInput")
    idx_d = nc.dram_tensor("idxw", [P, nidxcol], i16, kind="ExternalInput")
    out_d = nc.dram_tensor("out", [P, nslot * LANES], bf16,
                           kind="ExternalOutput")

    with tile.TileContext(nc) as tc:
        with tc.tile_pool(name="b4", bufs=1) as bp, \
             tc.tile_pool(name="r12", bufs=1) as gp, \
             tc.tile_pool(name="ix", bufs=1) as ip:

            B4 = bp.tile([P, 4 * T * LANES], bf16, tag="b4", name="B4")
            idxt = ip.tile([P, nidxcol], i16, tag="idx", name="idxt")
            nc.sync.dma_start(idxt[:], idx_d[:])
            b4 = B4[:]

            def pdim(ap):
                return [list(p) for p in ap.ap][0]

            REG = T * LANES
            # init region tails the level builds leave unwritten
            for r in (1, 2, 3):
                nc.vector.memset(
                    b4[:, r * REG + (T - 128) * LANES:(r + 1) * REG], 0)

            # input -> region 0, chunked so l1 can start early
            ndma = 4
            dstep = REG // ndma
            for j in range(ndma):
                nc.sync.dma_start(b4[:, j * dstep:(j + 1) * dstep],
                                  xb_d[:, j * dstep:(j + 1) * dstep])

            sizes = {}
            for key, c in plan:
                sizes[key] = sizes.get(key, 0) + c
            state = {"slot": 0, "col": 0}
            pending = []

            def emit_set(keys, win):
                # one ap_gather over a 2-region window (regions win, win+1)
                n = sum(sizes[k] for k in keys)
                n_idx = 2 * n
                r12 = gp.tile([P, n_idx * LANES], bf16,
                              tag=f"r12_{state['slot']}",
                              name=f"r12_{state['slot']}")
                src3 = AP(b4.tensor, b4.offset + win * REG,
                          [pdim(b4), [LANES, 2 * T], [1, LANES]])
                dst3 = AP(r12[:].tensor, r12[:].offset,
                          [pdim(r12[:]), [LANES, n_idx], [1, LANES]])
                co = state["col"]
                nc.gpsimd.ap_gather(
                    dst3, src3, idxt[:, co:co + n_idx // 16],
                    channels=P, num_elems=2 * T, d=LANES, num_idxs=n_idx)
                pending.append((r12, n, state["slot"]))
                state["slot"] += n
                state["col"] += n_idx // 16

            def emit_fold(item):
                # pairwise max folded IN PLACE into the r12 prefix
                r12, n, off = item
                v = r12[:]
                in0 = AP(v.tensor, v.offset,
                         [pdim(v), [2 * LANES, n], [1, LANES]])
                in1 = AP(v.tensor, v.offset + LANES,
                         [pdim(v), [2 * LANES, n], [1, LANES]])
                dst = AP(v.tensor, v.offset,
                         [pdim(v), [LANES, n], [1, LANES]])
                nc.vector.tensor_tensor(dst, in0, in1, MAX)
                out_src = AP(v.tensor, v.offset, [pdim(v), [1, n * LANES]])
                nc.sync.dma_start(
                    out_d[:, off * LANES:(off + n) * LANES], out_src)

            def build(dst_off, s0_off, s1_off, w):
                nc.vector.tensor_tensor(
                    b4[:, dst_off:dst_off + w * LANES],
                    b4[:, s0_off:s0_off + w * LANES],
                    b4[:, s1_off:s1_off + w * LANES], MAX)

            # l1 (region 0 -> 1), col-chunked behind the DMA chunks
            step = T // ndma
            for j in range(ndma):
                a = j * step
                bb = min((j + 1) * step, T - 1)
                build(REG + a * LANES, a * LANES, (a + 1) * LANES, bb - a)
            emit_set([(0, 0), (1, 0)], 0)

            for kk in (2, 3):
                s = 1 << (kk - 1)
                build((kk % 4) * REG, ((kk - 1) % 4) * REG,
                      ((kk - 1) % 4) * REG + s * LANES, T - 2 * s + 1)
            emit_set([(2, 0), (3, 0)], 2)

            for kk in (4, 5):
                s = 1 << (kk - 1)
                build((kk % 4) * REG, ((kk - 1) % 4) * REG,
                      ((kk - 1) % 4) * REG + s * LANES, T - 2 * s + 1)
            emit_set([(4, 0), (5, 0)], 0)

            # l6 ranges -> regions 2, 3 (src l5 = region 1)
            for _, blo, bhi, drow in PS_RANGES[6]:
                build(drow * REG, REG + blo * LANES,
                      REG + (blo + 32) * LANES, bhi - blo)
            emit_set([(6, 0), (6, 1)], 2)

            # l7 ranges -> regions 0, 1 (src l6 sub-tiles at local origin)
            build(0, 2 * REG, 2 * REG + 64 * LANES, 1152)
            build(REG, 3 * REG, 3 * REG + 64 * LANES, 897)
            emit_set([(7, 0), (7, 1)], 0)

            for item in pending:
                emit_fold(item)

    nc.compile()
    return nc


# ---------------------------------------------------------------- kernel
def _prepare(segments, max_len):
    seg0 = np.asarray(segments, dtype=np.float32)[0]
    (lo_s, hi_s), (lo_e, hi_e) = _windows(seg0, int(max_len))
    lev_s = _plan_half(lo_s, hi_s)
    lev_e = _plan_half(lo_e, hi_e)
    plan = _uniform_plan(lev_s, lev_e)
    idx_s, qmap_s = _half_streams(lev_s, plan)
    idx_e, qmap_e = _half_streams(lev_e, plan)
    return plan, (idx_s, qmap_s), (idx_e, qmap_e)


def kernel(feature, segments, max_len=T, **_unused):
    from concourse import bass_utils

    feature = np.asarray(feature)
    assert feature.shape == (B, C2, T), feature.shape
    plan, (idx_s, qmap_s), (idx_e, qmap_e) = _prepare(segments, int(max_len))

    key = ("prog", tuple(plan))
    if key not in _CACHE:
        _CACHE[key] = _build_program(plan)
    nc = _CACHE[key]

    feat_bf = feature.astype(BF16)
    in_maps = []
    for k in range(N_CORES):
        xc = feat_bf[:, k * P:(k + 1) * P, :]           # [B, 128, T]
        xb = np.ascontiguousarray(
            xc.transpose(1, 2, 0)).reshape(P, T * LANES)  # [c, t, lane]
        idxw = idx_s if k < 4 else idx_e
        in_maps.append({"xb": xb, "idxw": idxw})
    _CACHE["last_in_maps"] = in_maps

    res = bass_utils.run_bass_kernel_spmd(
        nc, in_maps, core_ids=list(range(N_CORES)))

    nslot = sum(c for _, c in plan)
    out = np.empty((B, C2, T), dtype=np.float32)
    for k in range(N_CORES):
        v = np.asarray(res.results[k]["out"]).reshape(P, nslot, LANES)
        qmap = qmap_s if k < 4 else qmap_e
        oq = _unpermute(v, qmap)          # [P, T, LANES] float32
        out[:, k * P:(k + 1) * P, :] = oq.transpose(2, 0, 1)
    return out


def _unpermute(v, qmap):
    """v: [P, nslot, LANES] bf16; qmap: slot -> q (-1 pads, dup q twice)."""
    vf = v.astype(np.float32)
    out = np.empty((P, T, LANES), dtype=np.float32)
    sl = np.nonzero(qmap >= 0)[0]
    qs = qmap[sl]
    out[:, qs, :] = vf[:, sl, :]
    # k=8 queries occupy two slots with the same q: max-combine those few
    uq, first, cnt = np.unique(qs, return_index=True, return_counts=True)
    for q in uq[cnt > 1]:
        ss = sl[qs == q]
        out[:, q, :] = vf[:, ss, :].max(axis=1)
    return out
